# revision 29
# baseline (speedup 1.0000x reference)
"""Windowed attention (swin-style, 49-token windows, 8 heads) with DynamicPosBias.

Design (measured 232us vs 968-1283us baseline, rel err 4.9e-4):
- DynamicPosBias MLP + rel-index gather + exp() computed on HOST (tiny,
  replicated); device receives E = exp(rpb) as a [128, 392] f16 table and
  applies it as one elementwise multiply after exp(QK).
- Two windows stacked per 128 SBUF partitions (window A -> partitions 0-63,
  window B -> 64-127, head-dim K=64).  QK and PV matmuls for the two windows
  run as concurrent diagonal PE tiles at tile_position (0,0) / (64,64)
  (auto-derived from base partitions); vector/scalar ops run once per pair
  at ~full 128-partition width.
- All HBM tensors are partition-major [NGRP, 128, big-contiguous] so every
  DMA is a plain [128, multi-KB-lines] transfer (128 descriptors covering
  all 128 partitions = all 16 DMA engines fed), batched GRP=8 window-pairs
  per dma_start.  v/out rows 49-63/113-127 are padding; trading +13% bytes
  for full-width descriptors measured FASTER than unpadded 49-row APs.
- qk loads on the sync HWDGE ring; v loads + out stores on the scalar ring
  (queue diversity keeps the 16 SDMA engines fed from two rings).
- Output written f16 (cast to f32 on host), halving output HBM traffic.
- Software-pipelined with LAG=2 pairs between QK and PV stages.
"""

import numpy as np
from contextlib import ExitStack

import concourse.bass as bass
import concourse.mybir as mybir
import concourse.tile as tile
from concourse import bacc
from concourse.bass_utils import run_bass_kernel_spmd

G = 7
NTOK = 49           # tokens per window
H = 8               # heads
HD = 64             # head dim
C = 512
B = 2048
NCORES = 8
W = B // NCORES     # windows per core (256)
NPAIR = W // 2      # window pairs per core (128)
GRP = 8             # window pairs per DMA group
NGRP = NPAIR // GRP
ELAG = 2            # pairs between QK and exp emission (keeps ACT waits short)
PLAG = 3            # additional pairs between exp and PV (total QK->PV lag 5)
QCOLS = H * NTOK    # 392
VCOLS = H * (HD + 1)  # 520
F32 = mybir.dt.float32
F16 = mybir.dt.float16


def _rel_idx():
    coords = np.stack(np.meshgrid(np.arange(G), np.arange(G), indexing="ij")).reshape(2, -1)
    rel = (coords[:, :, None] - coords[:, None, :]).transpose(1, 2, 0)
    rel = rel.copy()
    rel[:, :, 0] += G - 1
    rel[:, :, 1] += G - 1
    rel[:, :, 0] *= 2 * G - 1
    return rel.sum(-1)  # [i, j] in [0, 169)


def _host_pos_bias(pos_proj_w, pos_proj_b, ln1_g, ln1_b, w1, b1,
                   ln2_g, ln2_b, w2, b2, ln3_g, ln3_b, w3, b3):
    """Full DynamicPosBias MLP on host -> rpb [H, 49, 49] f32."""
    pb = np.arange(1 - G, G, dtype=np.float32)
    biases = np.stack(np.meshgrid(pb, pb, indexing="ij")).reshape(2, -1).T  # [169, 2]

    def ln(x, g, b, eps=1e-5):
        mu = x.mean(-1, keepdims=True)
        var = ((x - mu) ** 2).mean(-1, keepdims=True)
        return (x - mu) / np.sqrt(var + eps) * g + b

    pos = biases @ pos_proj_w + pos_proj_b
    pos = np.maximum(ln(pos, ln1_g, ln1_b), 0.0) @ w1 + b1
    pos = np.maximum(ln(pos, ln2_g, ln2_b), 0.0) @ w2 + b2
    pos = np.maximum(ln(pos, ln3_g, ln3_b), 0.0) @ w3 + b3  # [169, H]
    rpb = pos[_rel_idx()]          # [i, j, H]
    return rpb.transpose(2, 0, 1)  # [H, i, j]


_CACHED_NC = None
LAST_RESULTS = None


def _build_nc():
    global _CACHED_NC
    if _CACHED_NC is not None:
        return _CACHED_NC
    nc = bacc.Bacc(None, target_bir_lowering=False)

    qk_d = nc.dram_tensor("qk", [NGRP, 128, GRP * 2 * QCOLS], F16, kind="ExternalInput")
    v_d = nc.dram_tensor("vaug", [NGRP, 128, GRP * VCOLS], F16, kind="ExternalInput")
    ef_d = nc.dram_tensor("efull", [128, QCOLS], F16, kind="ExternalInput")
    out_d = nc.dram_tensor("out", [NGRP, 128, GRP * C], F16, kind="ExternalOutput")

    with tile.TileContext(nc) as tc, ExitStack() as ctx:
        const = ctx.enter_context(tc.tile_pool(name="const", bufs=1))
        qkpool = ctx.enter_context(tc.tile_pool(name="qkpool", bufs=5))
        vpool = ctx.enter_context(tc.tile_pool(name="vpool", bufs=5))
        opool = ctx.enter_context(tc.tile_pool(name="opool", bufs=3))
        expool = ctx.enter_context(tc.tile_pool(name="expool", bufs=3))
        exbpool = ctx.enter_context(tc.tile_pool(name="exbpool", bufs=PLAG + 2))
        recpool = ctx.enter_context(tc.tile_pool(name="recpool", bufs=3))
        stps = ctx.enter_context(tc.tile_pool(name="stps", bufs=3, space="PSUM"))
        pvps = ctx.enter_context(tc.tile_pool(name="pvps", bufs=2, space="PSUM"))

        ef_t = const.tile([128, QCOLS], F16, tag="ef")
        nc.sync.dma_start(ef_t[:], ef_d[:])

        st_stash = {}  # pair index -> (st, v_t, o_t, p_in_group, grp)
        stash = {}     # pair index -> (exb, v_t, o_t, p_in_group, grp)
        o_last = {}    # grp -> o_t tile

        def do_exp(gp):
            # emitted ELAG pairs after the QK matmuls so the ACT-queue wait is
            # already satisfied and never head-of-line-blocks the DMA issues
            # that share the scalar sequencer
            st, v_t, o_t, p, g = st_stash.pop(gp)
            ex = expool.tile([128, QCOLS], F16, tag="ex")
            nc.scalar.activation(ex[:], st[:], mybir.ActivationFunctionType.Exp)
            exb = exbpool.tile([128, QCOLS], F16, tag="exb")
            nc.gpsimd.tensor_tensor(out=exb[:], in0=ex[:], in1=ef_t[:],
                                    op=mybir.AluOpType.mult)
            stash[gp] = (exb, v_t, o_t, p, g)

        def do_pv(gp):
            exb, v_t, o_t, p, g = stash.pop(gp)
            pv0 = pvps.tile([128, 4 * 65], F32, tag="pv0")
            pv1 = pvps.tile([128, 4 * 65], F32, tag="pv1")
            for h in range(H):
                dst = pv0 if h < 4 else pv1
                m = h % 4
                for b_ in (0, 64):
                    nc.tensor.matmul(
                        out=dst[b_:b_ + NTOK, 65 * m:65 * (m + 1)],
                        lhsT=exb[b_:b_ + NTOK, NTOK * h:NTOK * (h + 1)],
                        rhs=v_t[b_:b_ + NTOK, VCOLS * p + 65 * h:VCOLS * p + 65 * (h + 1)],
                        start=True, stop=True,
                    )
            rec = recpool.tile([128, H], F32, tag="rec")
            nc.vector.reciprocal(
                rec[:, 0:4].rearrange("p (h o) -> p h o", o=1),
                pv0[:].rearrange("p (h c) -> p h c", c=65)[:, :, 64:65],
            )
            nc.vector.reciprocal(
                rec[:, 4:8].rearrange("p (h o) -> p h o", o=1),
                pv1[:].rearrange("p (h c) -> p h c", c=65)[:, :, 64:65],
            )
            for half, pv in ((0, pv0), (1, pv1)):
                nc.vector.tensor_tensor(
                    out=o_t[:, C * p + 256 * half:C * p + 256 * (half + 1)].rearrange(
                        "p (h c) -> p h c", c=HD),
                    in0=pv[:].rearrange("p (h c) -> p h c", c=65)[:, :, 0:HD],
                    in1=rec[:, 4 * half:4 * half + 4]
                    .rearrange("p (h o) -> p h o", o=1)
                    .to_broadcast([128, 4, HD]),
                    op=mybir.AluOpType.mult,
                )
            if p == GRP // 2 - 1:
                nc.scalar.dma_start(
                    out_d[g][:, 0:GRP // 2 * C], o_t[:, 0:GRP // 2 * C])
            if p == GRP - 1:
                nc.scalar.dma_start(
                    out_d[g][:, GRP // 2 * C:], o_t[:, GRP // 2 * C:])
                o_last.pop(g, None)

        for g in range(NGRP):
            qk_t = qkpool.tile([128, GRP * 2 * QCOLS], F16, tag="qk")
            half = GRP * QCOLS  # split load so the first pairs' data lands sooner
            nc.sync.dma_start(qk_t[:, 0:half], qk_d[g][:, 0:half])
            nc.sync.dma_start(qk_t[:, half:], qk_d[g][:, half:])
            v_t = vpool.tile([128, GRP * VCOLS], F16, tag="v")
            nc.scalar.dma_start(v_t[:], v_d[g])
            o_t = opool.tile([128, GRP * C], F16, tag="o")
            o_last[g] = o_t
            for p in range(GRP):
                gp = g * GRP + p
                qbase = p * 2 * QCOLS
                kbase = qbase + QCOLS
                st = stps.tile([128, QCOLS], F32, tag="st")
                for h in range(H):
                    for b_ in (0, 64):
                        nc.tensor.matmul(
                            out=st[b_:b_ + NTOK, NTOK * h:NTOK * (h + 1)],
                            lhsT=qk_t[b_:b_ + HD, kbase + NTOK * h:kbase + NTOK * (h + 1)],
                            rhs=qk_t[b_:b_ + HD, qbase + NTOK * h:qbase + NTOK * (h + 1)],
                            start=True, stop=True,
                        )
                st_stash[gp] = (st, v_t, o_t, p, g)
                if gp >= ELAG:
                    do_exp(gp - ELAG)
                if gp >= ELAG + PLAG:
                    do_pv(gp - ELAG - PLAG)
        for gp in sorted(st_stash.keys()):
            do_exp(gp)
        for gp in sorted(stash.keys()):
            do_pv(gp)

    nc.finalize()
    _CACHED_NC = nc
    return nc


def kernel(q, k, v, pos_proj_w, pos_proj_b, ln1_g, ln1_b, w1, b1,
           ln2_g, ln2_b, w2, b2, ln3_g, ln3_b, w3, b3):
    q = np.asarray(q, dtype=np.float32)
    k = np.asarray(k, dtype=np.float32)
    v = np.asarray(v, dtype=np.float32)

    # host: q pre-scaled by 1/sqrt(hd); per-window transposed layout [Bw, 64, 392]
    qt_all = np.ascontiguousarray(
        (q * np.float32(HD ** -0.5)).astype(np.float16)
        .reshape(B, NTOK, H, HD).transpose(0, 3, 2, 1)).reshape(B, HD, QCOLS)
    kt_all = np.ascontiguousarray(
        k.astype(np.float16).reshape(B, NTOK, H, HD).transpose(0, 3, 2, 1)
    ).reshape(B, HD, QCOLS)

    # v with fused ones column: [Bw, 49, 520]
    va = np.empty((B, NTOK, H, HD + 1), np.float16)
    va[..., :HD] = v.reshape(B, NTOK, H, HD)
    va[..., HD] = 1.0

    # host DynamicPosBias -> E = exp(rpb), duplicated on partition blocks
    rpb = _host_pos_bias(
        np.asarray(pos_proj_w, np.float32), np.asarray(pos_proj_b, np.float32),
        np.asarray(ln1_g, np.float32), np.asarray(ln1_b, np.float32),
        np.asarray(w1, np.float32), np.asarray(b1, np.float32),
        np.asarray(ln2_g, np.float32), np.asarray(ln2_b, np.float32),
        np.asarray(w2, np.float32), np.asarray(b2, np.float32),
        np.asarray(ln3_g, np.float32), np.asarray(ln3_b, np.float32),
        np.asarray(w3, np.float32), np.asarray(b3, np.float32),
    )  # [H, i, j]
    E = np.exp(rpb).transpose(2, 0, 1).reshape(NTOK, QCOLS)  # [j, h*49+i]
    efull = np.zeros((128, QCOLS), np.float16)
    efull[0:NTOK] = E
    efull[64:64 + NTOK] = E

    in_maps = []
    for c in range(NCORES):
        sl = slice(c * W, (c + 1) * W)
        # qk: [NGRP, 128(b*64+d), GRP, 2(q|k), 392]
        qtc = qt_all[sl].reshape(NGRP, GRP, 2, HD, QCOLS)
        ktc = kt_all[sl].reshape(NGRP, GRP, 2, HD, QCOLS)
        qk = np.empty((NGRP, 2, HD, GRP, 2, QCOLS), np.float16)
        qk[:, :, :, :, 0, :] = qtc.transpose(0, 2, 3, 1, 4)
        qk[:, :, :, :, 1, :] = ktc.transpose(0, 2, 3, 1, 4)
        qk = qk.reshape(NGRP, 128, GRP * 2 * QCOLS)

        # v: [NGRP, 128(b*64+j, rows 49-63 zero), GRP, 520]
        vz = np.zeros((NGRP, 2, HD, GRP, VCOLS), np.float16)
        vz[:, :, :NTOK] = va[sl].reshape(NGRP, GRP, 2, NTOK, VCOLS).transpose(0, 2, 3, 1, 4)
        vz = vz.reshape(NGRP, 128, GRP * VCOLS)

        in_maps.append({"qk": qk, "vaug": vz, "efull": efull})

    nc = _build_nc()
    res = run_bass_kernel_spmd(nc, in_maps, core_ids=list(range(NCORES)))
    global LAST_RESULTS
    LAST_RESULTS = res

    out = np.empty((B, NTOK, C), np.float32)
    for c in range(NCORES):
        od = res.results[c]["out"]  # [NGRP, 128, GRP*512] f16
        od = od.reshape(NGRP, 2, HD, GRP, C).transpose(0, 3, 1, 2, 4)
        out[c * W:(c + 1) * W] = od.reshape(W, HD, C)[:, :NTOK, :].astype(np.float32)
    return out


# revision 30
# speedup vs baseline: 1.0842x; 1.0842x over previous
"""Windowed attention (swin-style, 49-token windows, 8 heads) with DynamicPosBias.

Design (measured 232us vs 968-1283us baseline, rel err 4.9e-4):
- DynamicPosBias MLP + rel-index gather + exp() computed on HOST (tiny,
  replicated); device receives E = exp(rpb) as a [128, 392] f16 table and
  applies it as one elementwise multiply after exp(QK).
- Two windows stacked per 128 SBUF partitions (window A -> partitions 0-63,
  window B -> 64-127, head-dim K=64).  QK and PV matmuls for the two windows
  run as concurrent diagonal PE tiles at tile_position (0,0) / (64,64)
  (auto-derived from base partitions); vector/scalar ops run once per pair
  at ~full 128-partition width.
- All HBM tensors are partition-major [NGRP, 128, big-contiguous] so every
  DMA is a plain [128, multi-KB-lines] transfer (128 descriptors covering
  all 128 partitions = all 16 DMA engines fed), batched GRP=8 window-pairs
  per dma_start.  v/out rows 49-63/113-127 are padding; trading +13% bytes
  for full-width descriptors measured FASTER than unpadded 49-row APs.
- qk loads on the sync HWDGE ring; v loads + out stores on the scalar ring
  (queue diversity keeps the 16 SDMA engines fed from two rings).
- Output written f16 (cast to f32 on host), halving output HBM traffic.
- Software-pipelined with LAG=2 pairs between QK and PV stages.
"""

import numpy as np
from contextlib import ExitStack

import concourse.bass as bass
import concourse.mybir as mybir
import concourse.tile as tile
from concourse import bacc
from concourse.bass_utils import run_bass_kernel_spmd

G = 7
NTOK = 49           # tokens per window
H = 8               # heads
HD = 64             # head dim
C = 512
B = 2048
NCORES = 8
W = B // NCORES     # windows per core (256)
NPAIR = W // 2      # window pairs per core (128)
GRP = 8             # window pairs per DMA group
NGRP = NPAIR // GRP
ELAG = 2            # pairs between QK and exp emission (keeps ACT waits short)
PLAG = 2            # additional pairs between exp and PV (total QK->PV lag 4)
QCOLS = H * NTOK    # 392
VCOLS = H * (HD + 1)  # 520
F32 = mybir.dt.float32
F16 = mybir.dt.float16


def _rel_idx():
    coords = np.stack(np.meshgrid(np.arange(G), np.arange(G), indexing="ij")).reshape(2, -1)
    rel = (coords[:, :, None] - coords[:, None, :]).transpose(1, 2, 0)
    rel = rel.copy()
    rel[:, :, 0] += G - 1
    rel[:, :, 1] += G - 1
    rel[:, :, 0] *= 2 * G - 1
    return rel.sum(-1)  # [i, j] in [0, 169)


def _host_pos_bias(pos_proj_w, pos_proj_b, ln1_g, ln1_b, w1, b1,
                   ln2_g, ln2_b, w2, b2, ln3_g, ln3_b, w3, b3):
    """Full DynamicPosBias MLP on host -> rpb [H, 49, 49] f32."""
    pb = np.arange(1 - G, G, dtype=np.float32)
    biases = np.stack(np.meshgrid(pb, pb, indexing="ij")).reshape(2, -1).T  # [169, 2]

    def ln(x, g, b, eps=1e-5):
        mu = x.mean(-1, keepdims=True)
        var = ((x - mu) ** 2).mean(-1, keepdims=True)
        return (x - mu) / np.sqrt(var + eps) * g + b

    pos = biases @ pos_proj_w + pos_proj_b
    pos = np.maximum(ln(pos, ln1_g, ln1_b), 0.0) @ w1 + b1
    pos = np.maximum(ln(pos, ln2_g, ln2_b), 0.0) @ w2 + b2
    pos = np.maximum(ln(pos, ln3_g, ln3_b), 0.0) @ w3 + b3  # [169, H]
    rpb = pos[_rel_idx()]          # [i, j, H]
    return rpb.transpose(2, 0, 1)  # [H, i, j]


_CACHED_NC = None
LAST_RESULTS = None


def _build_nc():
    global _CACHED_NC
    if _CACHED_NC is not None:
        return _CACHED_NC
    nc = bacc.Bacc(None, target_bir_lowering=False)

    qk_d = nc.dram_tensor("qk", [NGRP, 128, GRP * 2 * QCOLS], F16, kind="ExternalInput")
    v_d = nc.dram_tensor("vaug", [NGRP, 128, GRP * VCOLS], F16, kind="ExternalInput")
    ef_d = nc.dram_tensor("efull", [128, QCOLS], F16, kind="ExternalInput")
    out_d = nc.dram_tensor("out", [NGRP, 128, GRP * C], F16, kind="ExternalOutput")

    with tile.TileContext(nc) as tc, ExitStack() as ctx:
        const = ctx.enter_context(tc.tile_pool(name="const", bufs=1))
        qkpool = ctx.enter_context(tc.tile_pool(name="qkpool", bufs=4))
        vpool = ctx.enter_context(tc.tile_pool(name="vpool", bufs=4))
        opool = ctx.enter_context(tc.tile_pool(name="opool", bufs=3))
        expool = ctx.enter_context(tc.tile_pool(name="expool", bufs=3))
        exbpool = ctx.enter_context(tc.tile_pool(name="exbpool", bufs=PLAG + 2))
        recpool = ctx.enter_context(tc.tile_pool(name="recpool", bufs=3))
        stps = ctx.enter_context(tc.tile_pool(name="stps", bufs=3, space="PSUM"))
        pvps = ctx.enter_context(tc.tile_pool(name="pvps", bufs=2, space="PSUM"))

        ef_t = const.tile([128, QCOLS], F16, tag="ef")
        nc.sync.dma_start(ef_t[:], ef_d[:])

        st_stash = {}  # pair index -> (st, v_t, o_t, p_in_group, grp)
        stash = {}     # pair index -> (exb, v_t, o_t, p_in_group, grp)
        o_last = {}    # grp -> o_t tile

        def do_exp(gp):
            # emitted ELAG pairs after the QK matmuls so the ACT-queue wait is
            # already satisfied and never head-of-line-blocks the DMA issues
            # that share the scalar sequencer
            st, v_t, o_t, p, g = st_stash.pop(gp)
            ex = expool.tile([128, QCOLS], F16, tag="ex")
            nc.scalar.activation(ex[:], st[:], mybir.ActivationFunctionType.Exp)
            exb = exbpool.tile([128, QCOLS], F16, tag="exb")
            nc.gpsimd.tensor_tensor(out=exb[:], in0=ex[:], in1=ef_t[:],
                                    op=mybir.AluOpType.mult)
            stash[gp] = (exb, v_t, o_t, p, g)

        def do_pv(gp):
            exb, v_t, o_t, p, g = stash.pop(gp)
            pv0 = pvps.tile([128, 4 * 65], F32, tag="pv0")
            pv1 = pvps.tile([128, 4 * 65], F32, tag="pv1")
            for h in range(H):
                dst = pv0 if h < 4 else pv1
                m = h % 4
                for b_ in (0, 64):
                    nc.tensor.matmul(
                        out=dst[b_:b_ + NTOK, 65 * m:65 * (m + 1)],
                        lhsT=exb[b_:b_ + NTOK, NTOK * h:NTOK * (h + 1)],
                        rhs=v_t[b_:b_ + NTOK, VCOLS * p + 65 * h:VCOLS * p + 65 * (h + 1)],
                        start=True, stop=True,
                    )
            rec = recpool.tile([128, H], F32, tag="rec")
            nc.vector.reciprocal(
                rec[:, 0:4].rearrange("p (h o) -> p h o", o=1),
                pv0[:].rearrange("p (h c) -> p h c", c=65)[:, :, 64:65],
            )
            nc.vector.reciprocal(
                rec[:, 4:8].rearrange("p (h o) -> p h o", o=1),
                pv1[:].rearrange("p (h c) -> p h c", c=65)[:, :, 64:65],
            )
            for half, pv in ((0, pv0), (1, pv1)):
                nc.vector.tensor_tensor(
                    out=o_t[:, C * p + 256 * half:C * p + 256 * (half + 1)].rearrange(
                        "p (h c) -> p h c", c=HD),
                    in0=pv[:].rearrange("p (h c) -> p h c", c=65)[:, :, 0:HD],
                    in1=rec[:, 4 * half:4 * half + 4]
                    .rearrange("p (h o) -> p h o", o=1)
                    .to_broadcast([128, 4, HD]),
                    op=mybir.AluOpType.mult,
                )
            if p == GRP // 2 - 1:
                nc.scalar.dma_start(
                    out_d[g][:, 0:GRP // 2 * C], o_t[:, 0:GRP // 2 * C])
            if p == GRP - 1:
                nc.scalar.dma_start(
                    out_d[g][:, GRP // 2 * C:], o_t[:, GRP // 2 * C:])
                o_last.pop(g, None)

        for g in range(NGRP):
            qk_t = qkpool.tile([128, GRP * 2 * QCOLS], F16, tag="qk")
            nc.sync.dma_start(qk_t[:], qk_d[g])
            v_t = vpool.tile([128, GRP * VCOLS], F16, tag="v")
            nc.scalar.dma_start(v_t[:], v_d[g])
            o_t = opool.tile([128, GRP * C], F16, tag="o")
            o_last[g] = o_t
            for p in range(GRP):
                gp = g * GRP + p
                qbase = p * 2 * QCOLS
                kbase = qbase + QCOLS
                st = stps.tile([128, QCOLS], F32, tag="st")
                for h in range(H):
                    for b_ in (0, 64):
                        nc.tensor.matmul(
                            out=st[b_:b_ + NTOK, NTOK * h:NTOK * (h + 1)],
                            lhsT=qk_t[b_:b_ + HD, kbase + NTOK * h:kbase + NTOK * (h + 1)],
                            rhs=qk_t[b_:b_ + HD, qbase + NTOK * h:qbase + NTOK * (h + 1)],
                            start=True, stop=True,
                        )
                st_stash[gp] = (st, v_t, o_t, p, g)
                if gp >= ELAG:
                    do_exp(gp - ELAG)
                if gp >= ELAG + PLAG:
                    do_pv(gp - ELAG - PLAG)
        for gp in sorted(st_stash.keys()):
            do_exp(gp)
        for gp in sorted(stash.keys()):
            do_pv(gp)

    nc.finalize()
    _CACHED_NC = nc
    return nc


def kernel(q, k, v, pos_proj_w, pos_proj_b, ln1_g, ln1_b, w1, b1,
           ln2_g, ln2_b, w2, b2, ln3_g, ln3_b, w3, b3):
    q = np.asarray(q, dtype=np.float32)
    k = np.asarray(k, dtype=np.float32)
    v = np.asarray(v, dtype=np.float32)

    # host: q pre-scaled by 1/sqrt(hd); per-window transposed layout [Bw, 64, 392]
    qt_all = np.ascontiguousarray(
        (q * np.float32(HD ** -0.5)).astype(np.float16)
        .reshape(B, NTOK, H, HD).transpose(0, 3, 2, 1)).reshape(B, HD, QCOLS)
    kt_all = np.ascontiguousarray(
        k.astype(np.float16).reshape(B, NTOK, H, HD).transpose(0, 3, 2, 1)
    ).reshape(B, HD, QCOLS)

    # v with fused ones column: [Bw, 49, 520]
    va = np.empty((B, NTOK, H, HD + 1), np.float16)
    va[..., :HD] = v.reshape(B, NTOK, H, HD)
    va[..., HD] = 1.0

    # host DynamicPosBias -> E = exp(rpb), duplicated on partition blocks
    rpb = _host_pos_bias(
        np.asarray(pos_proj_w, np.float32), np.asarray(pos_proj_b, np.float32),
        np.asarray(ln1_g, np.float32), np.asarray(ln1_b, np.float32),
        np.asarray(w1, np.float32), np.asarray(b1, np.float32),
        np.asarray(ln2_g, np.float32), np.asarray(ln2_b, np.float32),
        np.asarray(w2, np.float32), np.asarray(b2, np.float32),
        np.asarray(ln3_g, np.float32), np.asarray(ln3_b, np.float32),
        np.asarray(w3, np.float32), np.asarray(b3, np.float32),
    )  # [H, i, j]
    E = np.exp(rpb).transpose(2, 0, 1).reshape(NTOK, QCOLS)  # [j, h*49+i]
    efull = np.zeros((128, QCOLS), np.float16)
    efull[0:NTOK] = E
    efull[64:64 + NTOK] = E

    in_maps = []
    for c in range(NCORES):
        sl = slice(c * W, (c + 1) * W)
        # qk: [NGRP, 128(b*64+d), GRP, 2(q|k), 392]
        qtc = qt_all[sl].reshape(NGRP, GRP, 2, HD, QCOLS)
        ktc = kt_all[sl].reshape(NGRP, GRP, 2, HD, QCOLS)
        qk = np.empty((NGRP, 2, HD, GRP, 2, QCOLS), np.float16)
        qk[:, :, :, :, 0, :] = qtc.transpose(0, 2, 3, 1, 4)
        qk[:, :, :, :, 1, :] = ktc.transpose(0, 2, 3, 1, 4)
        qk = qk.reshape(NGRP, 128, GRP * 2 * QCOLS)

        # v: [NGRP, 128(b*64+j, rows 49-63 zero), GRP, 520]
        vz = np.zeros((NGRP, 2, HD, GRP, VCOLS), np.float16)
        vz[:, :, :NTOK] = va[sl].reshape(NGRP, GRP, 2, NTOK, VCOLS).transpose(0, 2, 3, 1, 4)
        vz = vz.reshape(NGRP, 128, GRP * VCOLS)

        in_maps.append({"qk": qk, "vaug": vz, "efull": efull})

    nc = _build_nc()
    res = run_bass_kernel_spmd(nc, in_maps, core_ids=list(range(NCORES)))
    global LAST_RESULTS
    LAST_RESULTS = res

    out = np.empty((B, NTOK, C), np.float32)
    for c in range(NCORES):
        od = res.results[c]["out"]  # [NGRP, 128, GRP*512] f16
        od = od.reshape(NGRP, 2, HD, GRP, C).transpose(0, 3, 1, 2, 4)
        out[c * W:(c + 1) * W] = od.reshape(W, HD, C)[:, :NTOK, :].astype(np.float32)
    return out


# revision 33
# speedup vs baseline: 1.1238x; 1.0365x over previous
"""Windowed attention (swin-style, 49-token windows, 8 heads) with DynamicPosBias.

Design (measured 223-231us vs 968-1283us baseline, rel err 4.9e-4):
- DynamicPosBias MLP + rel-index gather + exp() computed on HOST (tiny,
  replicated); device receives E = exp(rpb) as a [128, 392] f16 table and
  applies it as one elementwise multiply after exp(QK).
- Two windows stacked per 128 SBUF partitions (window A -> partitions 0-63,
  window B -> 64-127, head-dim K=64).  QK and PV matmuls for the two windows
  run as concurrent diagonal PE tiles at tile_position (0,0) / (64,64)
  (auto-derived from base partitions); vector/scalar ops run once per pair
  at ~full 128-partition width.
- All HBM tensors are partition-major [NGRP, 128, big-contiguous] so every
  DMA is a plain [128, multi-KB-lines] transfer (128 descriptors covering
  all 128 partitions = all 16 DMA engines fed), batched GRP=8 window-pairs
  per dma_start.  v/out rows 49-63/113-127 are padding; trading +13% bytes
  for full-width descriptors measured FASTER than unpadded 49-row APs.
- qk loads on the sync HWDGE ring; v loads + out stores on the scalar ring
  (queue diversity keeps the 16 SDMA engines fed from two rings).
- Output written f16 (cast to f32 on host), halving output HBM traffic.
- The bias multiply runs on the otherwise-idle GpSimd engine, balancing the
  four compute engines at ~50-56% each under the DMA-bound span.
- Two-level software pipeline: exp emitted ELAG=2 pairs after its QK (so the
  ACT-queue wait is pre-satisfied and never head-of-line-blocks the DMA
  issues sharing the scalar sequencer), PV a further PLAG=2 pairs later.
"""

import numpy as np
from contextlib import ExitStack

import concourse.bass as bass
import concourse.mybir as mybir
import concourse.tile as tile
from concourse import bacc
from concourse.bass_utils import run_bass_kernel_spmd

G = 7
NTOK = 49           # tokens per window
H = 8               # heads
HD = 64             # head dim
C = 512
B = 2048
NCORES = 8
W = B // NCORES     # windows per core (256)
NPAIR = W // 2      # window pairs per core (128)
GRP = 8             # window pairs per DMA group
NGRP = NPAIR // GRP
ELAG = 2            # pairs between QK and exp emission (keeps ACT waits short)
PLAG = 2            # additional pairs between exp and PV (total QK->PV lag 4)
QCOLS = H * NTOK    # 392
VCOLS = H * (HD + 1)  # 520
F32 = mybir.dt.float32
F16 = mybir.dt.float16


def _rel_idx():
    coords = np.stack(np.meshgrid(np.arange(G), np.arange(G), indexing="ij")).reshape(2, -1)
    rel = (coords[:, :, None] - coords[:, None, :]).transpose(1, 2, 0)
    rel = rel.copy()
    rel[:, :, 0] += G - 1
    rel[:, :, 1] += G - 1
    rel[:, :, 0] *= 2 * G - 1
    return rel.sum(-1)  # [i, j] in [0, 169)


def _host_pos_bias(pos_proj_w, pos_proj_b, ln1_g, ln1_b, w1, b1,
                   ln2_g, ln2_b, w2, b2, ln3_g, ln3_b, w3, b3):
    """Full DynamicPosBias MLP on host -> rpb [H, 49, 49] f32."""
    pb = np.arange(1 - G, G, dtype=np.float32)
    biases = np.stack(np.meshgrid(pb, pb, indexing="ij")).reshape(2, -1).T  # [169, 2]

    def ln(x, g, b, eps=1e-5):
        mu = x.mean(-1, keepdims=True)
        var = ((x - mu) ** 2).mean(-1, keepdims=True)
        return (x - mu) / np.sqrt(var + eps) * g + b

    pos = biases @ pos_proj_w + pos_proj_b
    pos = np.maximum(ln(pos, ln1_g, ln1_b), 0.0) @ w1 + b1
    pos = np.maximum(ln(pos, ln2_g, ln2_b), 0.0) @ w2 + b2
    pos = np.maximum(ln(pos, ln3_g, ln3_b), 0.0) @ w3 + b3  # [169, H]
    rpb = pos[_rel_idx()]          # [i, j, H]
    return rpb.transpose(2, 0, 1)  # [H, i, j]


_CACHED_NC = None
LAST_RESULTS = None


def _build_nc():
    global _CACHED_NC
    if _CACHED_NC is not None:
        return _CACHED_NC
    nc = bacc.Bacc(None, target_bir_lowering=False)

    qk_d = nc.dram_tensor("qk", [NGRP, 128, GRP * 2 * QCOLS], F16, kind="ExternalInput")
    v_d = nc.dram_tensor("vaug", [NGRP, 128, GRP * VCOLS], F16, kind="ExternalInput")
    ef_d = nc.dram_tensor("efull", [128, QCOLS], F16, kind="ExternalInput")
    out_d = nc.dram_tensor("out", [NGRP, 128, GRP * C], F16, kind="ExternalOutput")

    with tile.TileContext(nc) as tc, ExitStack() as ctx:
        const = ctx.enter_context(tc.tile_pool(name="const", bufs=1))
        qkpool = ctx.enter_context(tc.tile_pool(name="qkpool", bufs=4))
        vpool = ctx.enter_context(tc.tile_pool(name="vpool", bufs=4))
        opool = ctx.enter_context(tc.tile_pool(name="opool", bufs=3))
        expool = ctx.enter_context(tc.tile_pool(name="expool", bufs=3))
        exbpool = ctx.enter_context(tc.tile_pool(name="exbpool", bufs=PLAG + 2))
        recpool = ctx.enter_context(tc.tile_pool(name="recpool", bufs=3))
        stps = ctx.enter_context(tc.tile_pool(name="stps", bufs=3, space="PSUM"))
        pvps = ctx.enter_context(tc.tile_pool(name="pvps", bufs=2, space="PSUM"))

        ef_t = const.tile([128, QCOLS], F16, tag="ef")
        nc.sync.dma_start(ef_t[:], ef_d[:])

        st_stash = {}  # pair index -> (st, v_t, o_t, p_in_group, grp)
        stash = {}     # pair index -> (exb, v_t, o_t, p_in_group, grp)
        o_last = {}    # grp -> o_t tile

        def do_exp(gp):
            # emitted ELAG pairs after the QK matmuls so the ACT-queue wait is
            # already satisfied and never head-of-line-blocks the DMA issues
            # that share the scalar sequencer
            st, v_t, o_t, p, g = st_stash.pop(gp)
            ex = expool.tile([128, QCOLS], F16, tag="ex")
            nc.scalar.activation(ex[:], st[:], mybir.ActivationFunctionType.Exp)
            exb = exbpool.tile([128, QCOLS], F16, tag="exb")
            nc.gpsimd.tensor_tensor(out=exb[:], in0=ex[:], in1=ef_t[:],
                                    op=mybir.AluOpType.mult)
            stash[gp] = (exb, v_t, o_t, p, g)

        def do_pv(gp):
            exb, v_t, o_t, p, g = stash.pop(gp)
            pv0 = pvps.tile([128, 4 * 65], F32, tag="pv0")
            pv1 = pvps.tile([128, 4 * 65], F32, tag="pv1")
            for h in range(H):
                dst = pv0 if h < 4 else pv1
                m = h % 4
                for b_ in (0, 64):
                    nc.tensor.matmul(
                        out=dst[b_:b_ + NTOK, 65 * m:65 * (m + 1)],
                        lhsT=exb[b_:b_ + NTOK, NTOK * h:NTOK * (h + 1)],
                        rhs=v_t[b_:b_ + NTOK, VCOLS * p + 65 * h:VCOLS * p + 65 * (h + 1)],
                        start=True, stop=True,
                    )
            rec = recpool.tile([128, H], F32, tag="rec")
            nc.vector.reciprocal(
                rec[:, 0:4].rearrange("p (h o) -> p h o", o=1),
                pv0[:].rearrange("p (h c) -> p h c", c=65)[:, :, 64:65],
            )
            nc.vector.reciprocal(
                rec[:, 4:8].rearrange("p (h o) -> p h o", o=1),
                pv1[:].rearrange("p (h c) -> p h c", c=65)[:, :, 64:65],
            )
            for half, pv in ((0, pv0), (1, pv1)):
                nc.vector.tensor_tensor(
                    out=o_t[:, C * p + 256 * half:C * p + 256 * (half + 1)].rearrange(
                        "p (h c) -> p h c", c=HD),
                    in0=pv[:].rearrange("p (h c) -> p h c", c=65)[:, :, 0:HD],
                    in1=rec[:, 4 * half:4 * half + 4]
                    .rearrange("p (h o) -> p h o", o=1)
                    .to_broadcast([128, 4, HD]),
                    op=mybir.AluOpType.mult,
                )
            if p == GRP // 2 - 1:
                nc.scalar.dma_start(
                    out_d[g][:, 0:GRP // 2 * C], o_t[:, 0:GRP // 2 * C])
            if p == GRP - 1:
                nc.scalar.dma_start(
                    out_d[g][:, GRP // 2 * C:], o_t[:, GRP // 2 * C:])
                o_last.pop(g, None)

        for g in range(NGRP):
            qk_t = qkpool.tile([128, GRP * 2 * QCOLS], F16, tag="qk")
            nc.sync.dma_start(qk_t[:], qk_d[g])
            v_t = vpool.tile([128, GRP * VCOLS], F16, tag="v")
            nc.sync.dma_start(v_t[:], v_d[g])
            o_t = opool.tile([128, GRP * C], F16, tag="o")
            o_last[g] = o_t
            for p in range(GRP):
                gp = g * GRP + p
                qbase = p * 2 * QCOLS
                kbase = qbase + QCOLS
                st = stps.tile([128, QCOLS], F32, tag="st")
                for h in range(H):
                    for b_ in (0, 64):
                        nc.tensor.matmul(
                            out=st[b_:b_ + NTOK, NTOK * h:NTOK * (h + 1)],
                            lhsT=qk_t[b_:b_ + HD, kbase + NTOK * h:kbase + NTOK * (h + 1)],
                            rhs=qk_t[b_:b_ + HD, qbase + NTOK * h:qbase + NTOK * (h + 1)],
                            start=True, stop=True,
                        )
                st_stash[gp] = (st, v_t, o_t, p, g)
                if gp >= ELAG:
                    do_exp(gp - ELAG)
                if gp >= ELAG + PLAG:
                    do_pv(gp - ELAG - PLAG)
        for gp in sorted(st_stash.keys()):
            do_exp(gp)
        for gp in sorted(stash.keys()):
            do_pv(gp)

    nc.finalize()
    _CACHED_NC = nc
    return nc


def kernel(q, k, v, pos_proj_w, pos_proj_b, ln1_g, ln1_b, w1, b1,
           ln2_g, ln2_b, w2, b2, ln3_g, ln3_b, w3, b3):
    q = np.asarray(q, dtype=np.float32)
    k = np.asarray(k, dtype=np.float32)
    v = np.asarray(v, dtype=np.float32)

    # host: q pre-scaled by 1/sqrt(hd); per-window transposed layout [Bw, 64, 392]
    qt_all = np.ascontiguousarray(
        (q * np.float32(HD ** -0.5)).astype(np.float16)
        .reshape(B, NTOK, H, HD).transpose(0, 3, 2, 1)).reshape(B, HD, QCOLS)
    kt_all = np.ascontiguousarray(
        k.astype(np.float16).reshape(B, NTOK, H, HD).transpose(0, 3, 2, 1)
    ).reshape(B, HD, QCOLS)

    # v with fused ones column: [Bw, 49, 520]
    va = np.empty((B, NTOK, H, HD + 1), np.float16)
    va[..., :HD] = v.reshape(B, NTOK, H, HD)
    va[..., HD] = 1.0

    # host DynamicPosBias -> E = exp(rpb), duplicated on partition blocks
    rpb = _host_pos_bias(
        np.asarray(pos_proj_w, np.float32), np.asarray(pos_proj_b, np.float32),
        np.asarray(ln1_g, np.float32), np.asarray(ln1_b, np.float32),
        np.asarray(w1, np.float32), np.asarray(b1, np.float32),
        np.asarray(ln2_g, np.float32), np.asarray(ln2_b, np.float32),
        np.asarray(w2, np.float32), np.asarray(b2, np.float32),
        np.asarray(ln3_g, np.float32), np.asarray(ln3_b, np.float32),
        np.asarray(w3, np.float32), np.asarray(b3, np.float32),
    )  # [H, i, j]
    E = np.exp(rpb).transpose(2, 0, 1).reshape(NTOK, QCOLS)  # [j, h*49+i]
    efull = np.zeros((128, QCOLS), np.float16)
    efull[0:NTOK] = E
    efull[64:64 + NTOK] = E

    in_maps = []
    for c in range(NCORES):
        sl = slice(c * W, (c + 1) * W)
        # qk: [NGRP, 128(b*64+d), GRP, 2(q|k), 392]
        qtc = qt_all[sl].reshape(NGRP, GRP, 2, HD, QCOLS)
        ktc = kt_all[sl].reshape(NGRP, GRP, 2, HD, QCOLS)
        qk = np.empty((NGRP, 2, HD, GRP, 2, QCOLS), np.float16)
        qk[:, :, :, :, 0, :] = qtc.transpose(0, 2, 3, 1, 4)
        qk[:, :, :, :, 1, :] = ktc.transpose(0, 2, 3, 1, 4)
        qk = qk.reshape(NGRP, 128, GRP * 2 * QCOLS)

        # v: [NGRP, 128(b*64+j, rows 49-63 zero), GRP, 520]
        vz = np.zeros((NGRP, 2, HD, GRP, VCOLS), np.float16)
        vz[:, :, :NTOK] = va[sl].reshape(NGRP, GRP, 2, NTOK, VCOLS).transpose(0, 2, 3, 1, 4)
        vz = vz.reshape(NGRP, 128, GRP * VCOLS)

        in_maps.append({"qk": qk, "vaug": vz, "efull": efull})

    nc = _build_nc()
    res = run_bass_kernel_spmd(nc, in_maps, core_ids=list(range(NCORES)))
    global LAST_RESULTS
    LAST_RESULTS = res

    out = np.empty((B, NTOK, C), np.float32)
    for c in range(NCORES):
        od = res.results[c]["out"]  # [NGRP, 128, GRP*512] f16
        od = od.reshape(NGRP, 2, HD, GRP, C).transpose(0, 3, 1, 2, 4)
        out[c * W:(c + 1) * W] = od.reshape(W, HD, C)[:, :NTOK, :].astype(np.float32)
    return out
